# revision 1
# baseline (speedup 1.0000x reference)
"""Local/global multihead attention on 8 NeuronCores (Trainium2, Bass/Tile).

Sharding: core c = b*2 + hg  (b = batch 0..3, hg = head-group 0/1, 8 heads each).
Each core computes q/k/v projections for its 8 heads on its batch, head-local
attention (slot 0 runs a dense 2048-key path driven by a per-core mask so the
SPMD program is uniform: hg0's slot 0 is the true global head with an all-ones
mask, hg1's slot 0 is a local head with a band mask), banded attention with
narrowed tq windows for slots 1-7, and the output projection restricted to its
head-group columns of wo. Host sums the two head-group partials per batch and
adds bo + bv @ wo.T (valid because softmax rows sum to 1).

All matmul operands are bf16 (TensorE runs 1 cyc/row vs 4 for fp32); PSUM
accumulation is fp32 throughout.
"""
import numpy as np
import ml_dtypes

E, H, D, LK = 1024, 16, 64, 128
SCALE = D ** -0.5
B, N = 4, 2048
FG = 512          # features per head-group (8 heads * 64)
NCORES = 8

_cache = {}


def _bf16(a):
    return np.ascontiguousarray(a.astype(ml_dtypes.bfloat16))


def _build():
    import concourse.bacc as bacc
    import concourse.tile as tile
    import concourse.mybir as mybir
    from concourse.bass import ts

    dt = mybir.dt
    AF = mybir.ActivationFunctionType

    nc = bacc.Bacc("TRN2", target_bir_lowering=False, debug=False,
                   num_devices=NCORES)

    xT = nc.dram_tensor("xT", [E, N], dt.bfloat16, kind="ExternalInput")
    wqT = nc.dram_tensor("wqT", [E, FG], dt.bfloat16, kind="ExternalInput")
    wkT = nc.dram_tensor("wkT", [E, FG], dt.bfloat16, kind="ExternalInput")
    wvT = nc.dram_tensor("wvT", [E, FG], dt.bfloat16, kind="ExternalInput")
    woT = nc.dram_tensor("woT", [FG, E], dt.bfloat16, kind="ExternalInput")
    bqc = nc.dram_tensor("bqc", [128, 4], dt.float32, kind="ExternalInput")
    bkc = nc.dram_tensor("bkc", [128, 4], dt.float32, kind="ExternalInput")
    lmask = nc.dram_tensor("lmask", [128, 6 * 512], dt.bfloat16, kind="ExternalInput")
    gmask = nc.dram_tensor("gmask", [16, 128, N], dt.bfloat16, kind="ExternalInput")
    out = nc.dram_tensor("out", [N, E], dt.float32, kind="ExternalOutput")

    # narrowed tq windows per dj variant (delta = (dj-1)*128)
    WIN = [(0, 128), (0, 256), (0, 384), (128, 512), (256, 512), (384, 512)]

    with tile.TileContext(nc) as tc:
        with (
            tc.tile_pool(name="wts", bufs=1) as wts,
            tc.tile_pool(name="xp", bufs=1) as xp,
            tc.tile_pool(name="qkv", bufs=1) as qkv,
            tc.tile_pool(name="att", bufs=3) as att,
            tc.tile_pool(name="gm", bufs=2) as gm,
            tc.tile_pool(name="small", bufs=4) as small,
            tc.tile_pool(name="ps", bufs=2, space="PSUM") as psp,
            tc.tile_pool(name="av", bufs=1, space="PSUM") as avp,
        ):
            # ---- load weights/x/masks ----
            xT_t = [xp.tile([128, N], dt.bfloat16, name=f"xT{i}", tag=f"xT{i}") for i in range(8)]
            for ec in range(8):
                nc.sync.dma_start(xT_t[ec][:], xT[ts(ec, 128), :])
            wq_t = [wts.tile([128, FG], dt.bfloat16, name=f"wq{i}", tag=f"wq{i}") for i in range(8)]
            wk_t = [wts.tile([128, FG], dt.bfloat16, name=f"wk{i}", tag=f"wk{i}") for i in range(8)]
            wv_t = [wts.tile([128, FG], dt.bfloat16, name=f"wv{i}", tag=f"wv{i}") for i in range(8)]
            for ec in range(8):
                nc.sync.dma_start(wq_t[ec][:], wqT[ts(ec, 128), :])
                nc.sync.dma_start(wk_t[ec][:], wkT[ts(ec, 128), :])
                nc.sync.dma_start(wv_t[ec][:], wvT[ts(ec, 128), :])
            wo_t = [wts.tile([128, E], dt.bfloat16, name=f"wo{i}", tag=f"wo{i}") for i in range(4)]
            for fc in range(4):
                nc.sync.dma_start(wo_t[fc][:], woT[ts(fc, 128), :])
            bq_t = small.tile([128, 4], dt.float32, name="bq", tag="bq")
            bk_t = small.tile([128, 4], dt.float32, name="bk", tag="bk")
            nc.sync.dma_start(bq_t[:], bqc[:, :])
            nc.sync.dma_start(bk_t[:], bkc[:, :])
            lm_t = wts.tile([128, 6 * 512], dt.bfloat16, name="lm", tag="lm")
            nc.sync.dma_start(lm_t[:], lmask[:, :])

            # ---- projections ----
            qT_sb = [qkv.tile([128, N], dt.bfloat16, name=f"qT{i}", tag=f"qT{i}") for i in range(4)]
            kT_sb = [qkv.tile([128, N], dt.bfloat16, name=f"kT{i}", tag=f"kT{i}") for i in range(4)]
            for dst, w_t, b_t in ((qT_sb, wq_t, bq_t), (kT_sb, wk_t, bk_t)):
                for fc in range(4):
                    for tcn in range(4):
                        ps = psp.tile([128, 512], dt.float32, name="ps", tag="ps")
                        for ec in range(8):
                            nc.tensor.matmul(
                                ps[:], w_t[ec][:, ts(fc, 128)],
                                xT_t[ec][:, ts(tcn, 512)],
                                start=(ec == 0), stop=(ec == 7))
                        nc.vector.tensor_scalar_add(
                            dst[fc][:, ts(tcn, 512)], ps[:], b_t[:, fc:fc + 1])
            # v natural layout, per-head 72-col strided tiles with ones col
            v_sb = [qkv.tile([128, 8 * 72], dt.bfloat16, name=f"v{i}", tag=f"v{i}") for i in range(16)]
            for tcn in range(16):
                ps = psp.tile([128, 512], dt.float32, name="ps", tag="ps")
                for ec in range(8):
                    nc.tensor.matmul(ps[:], xT_t[ec][:, ts(tcn, 128)], wv_t[ec][:],
                                     start=(ec == 0), stop=(ec == 7))
                src = ps[:].rearrange("p (h d) -> p h d", h=8)
                dst = v_sb[tcn][:].rearrange("p (h d) -> p h d", h=8)[:, :, 0:64]
                nc.vector.tensor_copy(dst, src)
                ones = v_sb[tcn][:].rearrange("p (h d) -> p h d", h=8)[:, :, 64:65]
                nc.vector.memset(ones, 1.0)

            outTn = [qkv.tile([128, N], dt.bfloat16, name=f"outTn{i}", tag=f"outTn{i}") for i in range(4)]

            def head_rows(t, h):
                r0 = (h % 2) * 64
                return t[h // 2][r0:r0 + 64, :]

            # ---- slot 0: dense 2048-key path with gmask (jc outer so each
            # gmask chunk is DMA'd once; 4 super psums accumulate in parallel)
            h = 0
            qh = head_rows(qT_sb, h)
            kh = head_rows(kT_sb, h)
            av_g = [avp.tile([128, 512], dt.float32, name=f"avg{i}", tag=f"avg{i}") for i in range(4)]
            for jc in range(16):
                gt = gm.tile([128, N], dt.bfloat16, name="gm", tag="gm")
                nc.sync.dma_start(gt[:], gmask[jc, :, :])
                for s in range(4):
                    ps = psp.tile([128, 512], dt.float32, name="ps", tag="ps")
                    nc.tensor.matmul(ps[:], kh[:, ts(jc, 128)], qh[:, ts(s, 512)],
                                     start=True, stop=True)
                    at = att.tile([128, 512], dt.bfloat16, name="at", tag="at")
                    nc.scalar.activation(at[:], ps[:], AF.Exp, scale=float(SCALE))
                    nc.vector.tensor_mul(at[:], at[:], gt[:, ts(s, 512)])
                    nc.tensor.matmul(
                        av_g[s][0:65, :], v_sb[jc][:, h * 72:h * 72 + 65],
                        at[:], start=(jc == 0), stop=(jc == 15),
                        skip_group_check=True)
            for s in range(4):
                rec = small.tile([1, 512], dt.float32, name="rec", tag="rec")
                nc.vector.reciprocal(rec[:], av_g[s][64:65, :])
                rec64 = small.tile([64, 512], dt.float32, name="rec64", tag="rec64")
                nc.gpsimd.partition_broadcast(rec64[:], rec[:])
                nc.vector.tensor_mul(head_rows(outTn, h)[:, ts(s, 512)],
                                     av_g[s][0:64, :], rec64[:])

            # ---- slots 1..7: banded path ----
            for h in range(1, 8):
                qh = head_rows(qT_sb, h)
                kh = head_rows(kT_sb, h)
                for s in range(4):
                    av = avp.tile([128, 512], dt.float32, name="av", tag="av", bufs=2)
                    djs = [dj for dj in range(6) if 0 <= s * 4 - 1 + dj <= 15]
                    for i, dj in enumerate(djs):
                        jc = s * 4 - 1 + dj
                        c0, c1 = WIN[dj]
                        ps = psp.tile([128, 512], dt.float32, name="ps", tag="ps")
                        nc.tensor.matmul(ps[:, c0:c1], kh[:, ts(jc, 128)],
                                         qh[:, s * 512 + c0:s * 512 + c1],
                                         start=True, stop=True)
                        at = att.tile([128, 512], dt.bfloat16, name="at", tag="at")
                        nc.scalar.activation(at[:, c0:c1], ps[:, c0:c1], AF.Exp,
                                             scale=float(SCALE))
                        nc.vector.tensor_mul(at[:, c0:c1], at[:, c0:c1],
                                             lm_t[:, dj * 512 + c0:dj * 512 + c1])
                        nc.tensor.matmul(
                            av[0:65, c0:c1], v_sb[jc][:, h * 72:h * 72 + 65],
                            at[:, c0:c1], start=(i == 0), stop=(i == len(djs) - 1),
                            skip_group_check=True)
                    rec = small.tile([1, 512], dt.float32, name="rec", tag="rec")
                    nc.vector.reciprocal(rec[:], av[64:65, :])
                    rec64 = small.tile([64, 512], dt.float32, name="rec64", tag="rec64")
                    nc.gpsimd.partition_broadcast(rec64[:], rec[:])
                    nc.vector.tensor_mul(head_rows(outTn, h)[:, ts(s, 512)],
                                         av[0:64, :], rec64[:])

            # ---- output projection ----
            for tcn in range(16):
                for oc in range(2):
                    ps = psp.tile([128, 512], dt.float32, name="ps", tag="ps")
                    for fc in range(4):
                        nc.tensor.matmul(ps[:], outTn[fc][:, ts(tcn, 128)],
                                         wo_t[fc][:, ts(oc, 512)],
                                         start=(fc == 0), stop=(fc == 3))
                    ob = att.tile([128, 512], dt.float32, name="ob", tag="ob")
                    nc.scalar.copy(ob[:], ps[:])
                    nc.sync.dma_start(out[ts(tcn, 128), ts(oc, 512)], ob[:])
    nc.finalize()
    return nc


def _host_inputs(x, wq, bq, wk, bk, wv, bv, wo, bo):
    """Build the 8 per-core input dicts."""
    r = np.arange(128)[:, None]
    c512 = np.arange(512)[None, :]
    lm = np.zeros((6, 128, 512), np.float32)
    for v in range(6):
        lm[v] = (np.abs((v - 1) * 128 + r - c512) <= LK)
    lm = _bf16(lm.transpose(1, 0, 2).reshape(128, 6 * 512))

    cN = np.arange(N)[None, :]
    gm_band = np.zeros((16, 128, N), np.float32)
    for jc in range(16):
        gm_band[jc] = (np.abs(128 * jc + r - cN) <= LK)
    gm_ones = _bf16(np.ones((16, 128, N), np.float32))
    gm_band = _bf16(gm_band)

    in_maps = []
    for core in range(NCORES):
        b, hg = core // 2, core % 2
        fsl = slice(hg * FG, (hg + 1) * FG)
        in_maps.append({
            "xT": _bf16(x[b].T),
            "wqT": _bf16(wq[fsl].T),
            "wkT": _bf16(wk[fsl].T),
            "wvT": _bf16(wv[fsl].T),
            "woT": _bf16(wo[:, fsl].T),
            "bqc": np.ascontiguousarray(bq[fsl].reshape(4, 128).T, np.float32),
            "bkc": np.ascontiguousarray(bk[fsl].reshape(4, 128).T, np.float32),
            "lmask": lm,
            "gmask": gm_ones if hg == 0 else gm_band,
        })
    return in_maps


def kernel(x, wq, bq, wk, bk, wv, bv, wo, bo):
    from concourse.bass_utils import run_bass_kernel_spmd

    x, wq, bq, wk, bk, wv, bv, wo, bo = (
        np.asarray(a, np.float32) for a in (x, wq, bq, wk, bk, wv, bv, wo, bo))

    if "nc" not in _cache:
        _cache["nc"] = _build()
    nc = _cache["nc"]

    in_maps = _host_inputs(x, wq, bq, wk, bk, wv, bv, wo, bo)
    res = run_bass_kernel_spmd(nc, in_maps, core_ids=list(range(NCORES)))
    _cache["last_results"] = res

    const = (bo + bv @ wo.T).astype(np.float32)        # [1024]
    out = np.empty((B, N, E), np.float32)
    for b in range(B):
        out[b] = res.results[2 * b]["out"] + res.results[2 * b + 1]["out"] + const
    return out



# revision 8
# speedup vs baseline: 1.0159x; 1.0159x over previous
"""Local/global multihead attention on 8 NeuronCores (Trainium2, Bass/Tile).

Sharding: core c = b*2 + hg  (b = batch 0..3, hg = head-group 0/1, 8 heads each).
Each core computes q/k/v projections for its 8 heads on its batch, head-local
attention (slot 0 runs a dense 2048-key path driven by a per-core mask so the
SPMD program is uniform: hg0's slot 0 is the true global head with an all-ones
mask, hg1's slot 0 is a local head with a band mask), banded attention with
narrowed tq windows for slots 1-7, and the output projection restricted to its
head-group columns of wo. Host sums the two head-group partials per batch and
adds bo + bv @ wo.T (valid because softmax rows sum to 1).

All matmul operands are bf16 (TensorE runs 1 cyc/row vs 4 for fp32); PSUM
accumulation is fp32 throughout.
"""
import numpy as np
import ml_dtypes

E, H, D, LK = 1024, 16, 64, 128
SCALE = D ** -0.5
B, N = 4, 2048
FG = 512          # features per head-group (8 heads * 64)
NCORES = 8

_cache = {}


def _bf16(a):
    return np.ascontiguousarray(a.astype(ml_dtypes.bfloat16))


def _build():
    import concourse.bacc as bacc
    import concourse.tile as tile
    import concourse.mybir as mybir
    from concourse.bass import ts

    dt = mybir.dt
    AF = mybir.ActivationFunctionType

    nc = bacc.Bacc("TRN2", target_bir_lowering=False, debug=False,
                   num_devices=NCORES)

    xT = nc.dram_tensor("xT", [E, N], dt.bfloat16, kind="ExternalInput")
    wqT = nc.dram_tensor("wqT", [E, FG], dt.bfloat16, kind="ExternalInput")
    wkT = nc.dram_tensor("wkT", [E, FG], dt.bfloat16, kind="ExternalInput")
    wvT = nc.dram_tensor("wvT", [E, FG], dt.bfloat16, kind="ExternalInput")
    woT = nc.dram_tensor("woT", [FG, E], dt.bfloat16, kind="ExternalInput")
    bqc = nc.dram_tensor("bqc", [128, 4], dt.float32, kind="ExternalInput")
    # strip0 [128, 3968]: slot-0 mask table. slice at 512s-128jc+1920 gives the
    # [128,512] mask for (jc, s): all-ones on hg0 (global head), band on hg1.
    # stripb [128, 384]: band-mask core for slots 1-7; slice at c-128dj+256.
    strip0 = nc.dram_tensor("strip0", [128, 3968], dt.bfloat16, kind="ExternalInput")
    stripb = nc.dram_tensor("stripb", [128, 384], dt.bfloat16, kind="ExternalInput")
    out = nc.dram_tensor("out", [N, E], dt.float32, kind="ExternalOutput")

    # narrowed tq windows per dj variant (delta = (dj-1)*128)
    WIN = [(0, 128), (0, 256), (0, 384), (128, 512), (256, 512), (384, 512)]

    with tile.TileContext(nc) as tc:
        with (
            tc.tile_pool(name="wts", bufs=1) as wts,
            tc.tile_pool(name="xp", bufs=1) as xp,
            tc.tile_pool(name="qkv", bufs=1) as qkv,
            tc.tile_pool(name="att", bufs=3) as att,
            tc.tile_pool(name="small", bufs=4) as small,
            tc.tile_pool(name="ps", bufs=2, space="PSUM") as psp,
            tc.tile_pool(name="av", bufs=1, space="PSUM") as avp,
        ):
            # ---- load weights/x/masks ----
            xT_t = [xp.tile([128, N], dt.bfloat16, name=f"xT{i}", tag=f"xT{i}") for i in range(8)]
            for ec in range(8):
                nc.sync.dma_start(xT_t[ec][:], xT[ts(ec, 128), :])
            wq_t = [wts.tile([128, FG], dt.bfloat16, name=f"wq{i}", tag=f"wq{i}") for i in range(8)]
            wk_t = [wts.tile([128, FG], dt.bfloat16, name=f"wk{i}", tag=f"wk{i}") for i in range(8)]
            wv_t = [wts.tile([128, FG], dt.bfloat16, name=f"wv{i}", tag=f"wv{i}") for i in range(8)]
            for ec in range(8):
                nc.sync.dma_start(wq_t[ec][:], wqT[ts(ec, 128), :])
                nc.sync.dma_start(wk_t[ec][:], wkT[ts(ec, 128), :])
                nc.sync.dma_start(wv_t[ec][:], wvT[ts(ec, 128), :])
            wo_t = [wts.tile([128, E], dt.bfloat16, name=f"wo{i}", tag=f"wo{i}") for i in range(4)]
            for fc in range(4):
                nc.sync.dma_start(wo_t[fc][:], woT[ts(fc, 128), :])
            bq_t = small.tile([128, 4], dt.float32, name="bq", tag="bq")
            nc.sync.dma_start(bq_t[:], bqc[:, :])
            s0_t = wts.tile([128, 3968], dt.bfloat16, name="s0", tag="s0")
            nc.sync.dma_start(s0_t[:], strip0[:, :])
            sb_t = wts.tile([128, 384], dt.bfloat16, name="sb", tag="sb")
            nc.sync.dma_start(sb_t[:], stripb[:, :])

            # ---- projections ----
            # k bias is dropped entirely: softmax is invariant to per-query
            # score offsets, and (q+bq)@bk only shifts each query's row.
            # q bias folds into the ScalarE PSUM evacuation (Identity+bias).
            qT_sb = [qkv.tile([128, N], dt.bfloat16, name=f"qT{i}", tag=f"qT{i}") for i in range(4)]
            kT_sb = [qkv.tile([128, N], dt.bfloat16, name=f"kT{i}", tag=f"kT{i}") for i in range(4)]
            for dst, w_t, b_t in ((qT_sb, wq_t, bq_t), (kT_sb, wk_t, None)):
                for fc in range(4):
                    for tcn in range(4):
                        ps = psp.tile([128, 512], dt.float32, name="ps", tag="ps")
                        for ec in range(8):
                            nc.tensor.matmul(
                                ps[:], w_t[ec][:, ts(fc, 128)],
                                xT_t[ec][:, ts(tcn, 512)],
                                start=(ec == 0), stop=(ec == 7))
                        if b_t is None:
                            nc.scalar.copy(dst[fc][:, ts(tcn, 512)], ps[:])
                        else:
                            nc.scalar.activation(
                                dst[fc][:, ts(tcn, 512)], ps[:], AF.Identity,
                                bias=b_t[:, fc:fc + 1])
            # v natural layout, per-head 72-col strided tiles with ones col
            v_sb = [qkv.tile([128, 8 * 72], dt.bfloat16, name=f"v{i}", tag=f"v{i}") for i in range(16)]
            for tcn in range(16):
                ps = psp.tile([128, 512], dt.float32, name="ps", tag="ps")
                for ec in range(8):
                    nc.tensor.matmul(ps[:], xT_t[ec][:, ts(tcn, 128)], wv_t[ec][:],
                                     start=(ec == 0), stop=(ec == 7))
                src = ps[:].rearrange("p (h d) -> p h d", h=8)
                dst = v_sb[tcn][:].rearrange("p (h d) -> p h d", h=8)[:, :, 0:64]
                nc.vector.tensor_copy(dst, src)
                ones = v_sb[tcn][:].rearrange("p (h d) -> p h d", h=8)[:, :, 64:65]
                nc.vector.memset(ones, 1.0)

            outTn = [qkv.tile([128, N], dt.bfloat16, name=f"outTn{i}", tag=f"outTn{i}") for i in range(4)]

            def head_rows(t, h):
                r0 = (h % 2) * 64
                return t[h // 2][r0:r0 + 64, :]

            # ---- slot 0: dense 2048-key path; mask tiles are static slices
            # of the strip0 table (no per-jc DMA). 4 super psums accumulate.
            h = 0
            qh = head_rows(qT_sb, h)
            kh = head_rows(kT_sb, h)
            av_g = [avp.tile([128, 512], dt.float32, name=f"avg{i}", tag=f"avg{i}") for i in range(4)]
            for jc in range(16):
                for s in range(4):
                    ps = psp.tile([128, 512], dt.float32, name="ps", tag="ps")
                    nc.tensor.matmul(ps[:], kh[:, ts(jc, 128)], qh[:, ts(s, 512)],
                                     start=True, stop=True)
                    at = att.tile([128, 512], dt.bfloat16, name="at", tag="at")
                    nc.scalar.activation(at[:], ps[:], AF.Exp, scale=float(SCALE))
                    off = 512 * s - 128 * jc + 1920
                    nc.vector.tensor_mul(at[:], at[:], s0_t[:, off:off + 512])
                    nc.tensor.matmul(
                        av_g[s][0:65, :], v_sb[jc][:, h * 72:h * 72 + 65],
                        at[:], start=(jc == 0), stop=(jc == 15),
                        skip_group_check=True)
            for s in range(4):
                rec = small.tile([1, 512], dt.float32, name="rec", tag="rec")
                nc.vector.reciprocal(rec[:], av_g[s][64:65, :])
                rec64 = small.tile([64, 512], dt.float32, name="rec64", tag="rec64")
                nc.gpsimd.partition_broadcast(rec64[:], rec[:])
                nc.vector.tensor_mul(head_rows(outTn, h)[:, ts(s, 512)],
                                     av_g[s][0:64, :], rec64[:])

            # ---- slots 1..7: banded path ----
            for h in range(1, 8):
                qh = head_rows(qT_sb, h)
                kh = head_rows(kT_sb, h)
                for s in range(4):
                    av = avp.tile([128, 512], dt.float32, name="av", tag="av", bufs=2)
                    djs = [dj for dj in range(6) if 0 <= s * 4 - 1 + dj <= 15]
                    for i, dj in enumerate(djs):
                        jc = s * 4 - 1 + dj
                        c0, c1 = WIN[dj]
                        ps = psp.tile([128, 512], dt.float32, name="ps", tag="ps")
                        nc.tensor.matmul(ps[:, c0:c1], kh[:, ts(jc, 128)],
                                         qh[:, s * 512 + c0:s * 512 + c1],
                                         start=True, stop=True)
                        at = att.tile([128, 512], dt.bfloat16, name="at", tag="at")
                        nc.scalar.activation(at[:, c0:c1], ps[:, c0:c1], AF.Exp,
                                             scale=float(SCALE))
                        u0 = c0 - 128 * dj + 256
                        nc.vector.tensor_mul(at[:, c0:c1], at[:, c0:c1],
                                             sb_t[:, u0:u0 + (c1 - c0)])
                        nc.tensor.matmul(
                            av[0:65, c0:c1], v_sb[jc][:, h * 72:h * 72 + 65],
                            at[:, c0:c1], start=(i == 0), stop=(i == len(djs) - 1),
                            skip_group_check=True)
                    rec = small.tile([1, 512], dt.float32, name="rec", tag="rec")
                    nc.vector.reciprocal(rec[:], av[64:65, :])
                    rec64 = small.tile([64, 512], dt.float32, name="rec64", tag="rec64")
                    nc.gpsimd.partition_broadcast(rec64[:], rec[:])
                    nc.vector.tensor_mul(head_rows(outTn, h)[:, ts(s, 512)],
                                         av[0:64, :], rec64[:])

            # ---- output projection ----
            for tcn in range(16):
                for oc in range(2):
                    ps = psp.tile([128, 512], dt.float32, name="ps", tag="ps")
                    for fc in range(4):
                        nc.tensor.matmul(ps[:], outTn[fc][:, ts(tcn, 128)],
                                         wo_t[fc][:, ts(oc, 512)],
                                         start=(fc == 0), stop=(fc == 3))
                    ob = att.tile([128, 512], dt.float32, name="ob", tag="ob")
                    nc.scalar.copy(ob[:], ps[:])
                    nc.sync.dma_start(out[ts(tcn, 128), ts(oc, 512)], ob[:])
    nc.finalize()
    return nc


def _host_inputs(x, wq, bq, wk, bk, wv, bv, wo, bo):
    """Build the 8 per-core input dicts."""
    r = np.arange(128)[:, None]
    # stripb [128, 384]: band core, stripb[r, u] = |r + 128 - u| <= LK
    u = np.arange(384)[None, :]
    stripb = _bf16((np.abs(r + 128 - u) <= LK).astype(np.float32))
    # strip0 [128, 3968]: band at |r + 1920 - c| <= LK (hg1) or all ones (hg0)
    c = np.arange(3968)[None, :]
    strip0_band = _bf16((np.abs(r + 1920 - c) <= LK).astype(np.float32))
    strip0_ones = _bf16(np.ones((128, 3968), np.float32))

    in_maps = []
    for core in range(NCORES):
        b, hg = core // 2, core % 2
        fsl = slice(hg * FG, (hg + 1) * FG)
        in_maps.append({
            "xT": _bf16(x[b].T),
            "wqT": _bf16(wq[fsl].T),
            "wkT": _bf16(wk[fsl].T),
            "wvT": _bf16(wv[fsl].T),
            "woT": _bf16(wo[:, fsl].T),
            "bqc": np.ascontiguousarray(bq[fsl].reshape(4, 128).T, np.float32),
            "strip0": strip0_ones if hg == 0 else strip0_band,
            "stripb": stripb,
        })
    return in_maps


def kernel(x, wq, bq, wk, bk, wv, bv, wo, bo):
    from concourse.bass_utils import run_bass_kernel_spmd

    x, wq, bq, wk, bk, wv, bv, wo, bo = (
        np.asarray(a, np.float32) for a in (x, wq, bq, wk, bk, wv, bv, wo, bo))

    if "nc" not in _cache:
        _cache["nc"] = _build()
    nc = _cache["nc"]

    in_maps = _host_inputs(x, wq, bq, wk, bk, wv, bv, wo, bo)
    res = run_bass_kernel_spmd(nc, in_maps, core_ids=list(range(NCORES)))
    _cache["last_results"] = res

    const = (bo + bv @ wo.T).astype(np.float32)        # [1024]
    out = np.empty((B, N, E), np.float32)
    for b in range(B):
        out[b] = res.results[2 * b]["out"] + res.results[2 * b + 1]["out"] + const
    return out



# revision 10
# speedup vs baseline: 1.2024x; 1.1836x over previous
"""Local/global multihead attention on 8 NeuronCores (Trainium2, Bass/Tile).

Sharding: core c = b*2 + hg  (b = batch 0..3, hg = head-group 0/1, 8 heads each).
Each core computes q/k/v projections for its 8 heads on its batch, head-local
attention (slot 0 runs a dense 2048-key path driven by a per-core mask so the
SPMD program is uniform: hg0's slot 0 is the true global head with an all-ones
mask, hg1's slot 0 is a local head with a band mask), banded attention with
narrowed tq windows for slots 1-7, and the output projection restricted to its
head-group columns of wo. Host sums the two head-group partials per batch and
adds bo + bv @ wo.T (valid because softmax rows sum to 1).

All matmul operands are bf16 (TensorE runs 1 cyc/row vs 4 for fp32); PSUM
accumulation is fp32 throughout.
"""
import numpy as np
import ml_dtypes

E, H, D, LK = 1024, 16, 64, 128
SCALE = D ** -0.5
B, N = 4, 2048
FG = 512          # features per head-group (8 heads * 64)
NCORES = 8

_cache = {}


def _bf16(a):
    return np.ascontiguousarray(a.astype(ml_dtypes.bfloat16))


def _build():
    import concourse.bacc as bacc
    import concourse.tile as tile
    import concourse.mybir as mybir
    from concourse.bass import ts

    dt = mybir.dt
    AF = mybir.ActivationFunctionType

    nc = bacc.Bacc("TRN2", target_bir_lowering=False, debug=False,
                   num_devices=NCORES)

    xT = nc.dram_tensor("xT", [E, N], dt.bfloat16, kind="ExternalInput")
    wqT = nc.dram_tensor("wqT", [E, FG], dt.bfloat16, kind="ExternalInput")
    wkT = nc.dram_tensor("wkT", [E, FG], dt.bfloat16, kind="ExternalInput")
    wvT = nc.dram_tensor("wvT", [E, FG], dt.bfloat16, kind="ExternalInput")
    woT = nc.dram_tensor("woT", [FG, E], dt.bfloat16, kind="ExternalInput")
    bqc = nc.dram_tensor("bqc", [128, 4], dt.float32, kind="ExternalInput")
    # strip0 [128, 3968]: slot-0 mask table. slice at 512s-128jc+1920 gives the
    # [128,512] mask for (jc, s): all-ones on hg0 (global head), band on hg1.
    # stripb [128, 384]: band-mask core for slots 1-7; slice at c-128dj+256.
    strip0 = nc.dram_tensor("strip0", [128, 3968], dt.bfloat16, kind="ExternalInput")
    stripb = nc.dram_tensor("stripb", [128, 384], dt.bfloat16, kind="ExternalInput")
    out = nc.dram_tensor("out", [N, E], dt.float32, kind="ExternalOutput")

    # narrowed tq windows per dj variant (delta = (dj-1)*128)
    WIN = [(0, 128), (0, 256), (0, 384), (128, 512), (256, 512), (384, 512)]

    with tile.TileContext(nc) as tc:
        with (
            tc.tile_pool(name="wts", bufs=1) as wts,
            tc.tile_pool(name="xp", bufs=1) as xp,
            tc.tile_pool(name="qkv", bufs=1) as qkv,
            tc.tile_pool(name="att", bufs=3) as att,
            tc.tile_pool(name="small", bufs=4) as small,
            tc.tile_pool(name="ps", bufs=2, space="PSUM") as psp,
            tc.tile_pool(name="av", bufs=1, space="PSUM") as avp,
        ):
            # ---- load weights/x/masks ----
            xT_t = [xp.tile([128, N], dt.bfloat16, name=f"xT{i}", tag=f"xT{i}") for i in range(8)]
            for ec in range(8):
                nc.sync.dma_start(xT_t[ec][:], xT[ts(ec, 128), :])
            wq_t = [wts.tile([128, FG], dt.bfloat16, name=f"wq{i}", tag=f"wq{i}") for i in range(8)]
            wk_t = [wts.tile([128, FG], dt.bfloat16, name=f"wk{i}", tag=f"wk{i}") for i in range(8)]
            wv_t = [wts.tile([128, FG], dt.bfloat16, name=f"wv{i}", tag=f"wv{i}") for i in range(8)]
            for ec in range(8):
                nc.sync.dma_start(wq_t[ec][:], wqT[ts(ec, 128), :])
                nc.sync.dma_start(wk_t[ec][:], wkT[ts(ec, 128), :])
                nc.sync.dma_start(wv_t[ec][:], wvT[ts(ec, 128), :])
            wo_t = [wts.tile([128, E], dt.bfloat16, name=f"wo{i}", tag=f"wo{i}") for i in range(4)]
            for fc in range(4):
                nc.sync.dma_start(wo_t[fc][:], woT[ts(fc, 128), :])
            bq_t = small.tile([128, 4], dt.float32, name="bq", tag="bq")
            nc.sync.dma_start(bq_t[:], bqc[:, :])
            s0_t = wts.tile([128, 3968], dt.bfloat16, name="s0", tag="s0")
            nc.sync.dma_start(s0_t[:], strip0[:, :])
            sb_t = wts.tile([128, 384], dt.bfloat16, name="sb", tag="sb")
            nc.sync.dma_start(sb_t[:], stripb[:, :])

            # ---- projections ----
            # k bias is dropped entirely: softmax is invariant to per-query
            # score offsets, and (q+bq)@bk only shifts each query's row.
            # q bias folds into the ScalarE PSUM evacuation (Identity+bias).
            qT_sb = [qkv.tile([128, N], dt.bfloat16, name=f"qT{i}", tag=f"qT{i}") for i in range(4)]
            kT_sb = [qkv.tile([128, N], dt.bfloat16, name=f"kT{i}", tag=f"kT{i}") for i in range(4)]
            for dst, w_t, b_t in ((qT_sb, wq_t, bq_t), (kT_sb, wk_t, None)):
                for fc in range(4):
                    for tcn in range(4):
                        ps = psp.tile([128, 512], dt.float32, name="ps", tag="ps")
                        for ec in range(8):
                            nc.tensor.matmul(
                                ps[:], w_t[ec][:, ts(fc, 128)],
                                xT_t[ec][:, ts(tcn, 512)],
                                start=(ec == 0), stop=(ec == 7))
                        if b_t is None:
                            nc.scalar.copy(dst[fc][:, ts(tcn, 512)], ps[:])
                        else:
                            nc.scalar.activation(
                                dst[fc][:, ts(tcn, 512)], ps[:], AF.Identity,
                                bias=b_t[:, fc:fc + 1])
            # v natural layout, per-head 72-col strided tiles with ones col
            v_sb = [qkv.tile([128, 8 * 72], dt.bfloat16, name=f"v{i}", tag=f"v{i}") for i in range(16)]
            for tcn in range(16):
                ps = psp.tile([128, 512], dt.float32, name="ps", tag="ps")
                for ec in range(8):
                    nc.tensor.matmul(ps[:], xT_t[ec][:, ts(tcn, 128)], wv_t[ec][:],
                                     start=(ec == 0), stop=(ec == 7))
                src = ps[:].rearrange("p (h d) -> p h d", h=8)
                dst = v_sb[tcn][:].rearrange("p (h d) -> p h d", h=8)[:, :, 0:64]
                nc.vector.tensor_copy(dst, src)
                ones = v_sb[tcn][:].rearrange("p (h d) -> p h d", h=8)[:, :, 64:65]
                nc.vector.memset(ones, 1.0)

            outTn = [qkv.tile([128, N], dt.bfloat16, name=f"outTn{i}", tag=f"outTn{i}") for i in range(4)]

            def head_rows(t, h):
                r0 = (h % 2) * 64
                return t[h // 2][r0:r0 + 64, :]

            # ---- slot 0: dense 2048-key path; mask tiles are static slices
            # of the strip0 table (no per-jc DMA). 4 super psums accumulate.
            h = 0
            qh = head_rows(qT_sb, h)
            kh = head_rows(kT_sb, h)
            av_g = [avp.tile([128, 512], dt.float32, name=f"avg{i}", tag=f"avg{i}") for i in range(4)]
            for jc in range(16):
                for s in range(4):
                    ps = psp.tile([128, 512], dt.float32, name="ps", tag="ps")
                    nc.tensor.matmul(ps[:], kh[:, ts(jc, 128)], qh[:, ts(s, 512)],
                                     start=True, stop=True)
                    at = att.tile([128, 512], dt.bfloat16, name="at", tag="at")
                    nc.scalar.activation(at[:], ps[:], AF.Exp, scale=float(SCALE))
                    off = 512 * s - 128 * jc + 1920
                    nc.vector.tensor_mul(at[:], at[:], s0_t[:, off:off + 512])
                    nc.tensor.matmul(
                        av_g[s][0:65, :], v_sb[jc][:, h * 72:h * 72 + 65],
                        at[:], start=(jc == 0), stop=(jc == 15),
                        skip_group_check=True)
            for s in range(4):
                # reciprocal_approx_fast mis-executes on PSUM inputs; stage the
                # denominator row through SBUF on ScalarE first.
                den = small.tile([1, 512], dt.float32, name="den", tag="den")
                nc.scalar.copy(den[:], av_g[s][64:65, :])
                rec = small.tile([1, 512], dt.float32, name="rec", tag="rec")
                nc.vector.reciprocal_approx_fast(rec[:], den[:])
                rec64 = small.tile([64, 512], dt.float32, name="rec64", tag="rec64")
                nc.gpsimd.partition_broadcast(rec64[:], rec[:])
                nc.vector.tensor_mul(head_rows(outTn, h)[:, ts(s, 512)],
                                     av_g[s][0:64, :], rec64[:])

            # ---- slots 1..7: banded path ----
            for h in range(1, 8):
                qh = head_rows(qT_sb, h)
                kh = head_rows(kT_sb, h)
                for s in range(4):
                    av = avp.tile([128, 512], dt.float32, name="av", tag="av", bufs=2)
                    djs = [dj for dj in range(6) if 0 <= s * 4 - 1 + dj <= 15]
                    for i, dj in enumerate(djs):
                        jc = s * 4 - 1 + dj
                        c0, c1 = WIN[dj]
                        ps = psp.tile([128, 512], dt.float32, name="ps", tag="ps")
                        nc.tensor.matmul(ps[:, c0:c1], kh[:, ts(jc, 128)],
                                         qh[:, s * 512 + c0:s * 512 + c1],
                                         start=True, stop=True)
                        at = att.tile([128, 512], dt.bfloat16, name="at", tag="at")
                        nc.scalar.activation(at[:, c0:c1], ps[:, c0:c1], AF.Exp,
                                             scale=float(SCALE))
                        u0 = c0 - 128 * dj + 256
                        nc.vector.tensor_mul(at[:, c0:c1], at[:, c0:c1],
                                             sb_t[:, u0:u0 + (c1 - c0)])
                        nc.tensor.matmul(
                            av[0:65, c0:c1], v_sb[jc][:, h * 72:h * 72 + 65],
                            at[:, c0:c1], start=(i == 0), stop=(i == len(djs) - 1),
                            skip_group_check=True)
                    den = small.tile([1, 512], dt.float32, name="den", tag="den")
                    nc.scalar.copy(den[:], av[64:65, :])
                    rec = small.tile([1, 512], dt.float32, name="rec", tag="rec")
                    nc.vector.reciprocal_approx_fast(rec[:], den[:])
                    rec64 = small.tile([64, 512], dt.float32, name="rec64", tag="rec64")
                    nc.gpsimd.partition_broadcast(rec64[:], rec[:])
                    nc.vector.tensor_mul(head_rows(outTn, h)[:, ts(s, 512)],
                                         av[0:64, :], rec64[:])

            # ---- output projection ----
            for tcn in range(16):
                for oc in range(2):
                    ps = psp.tile([128, 512], dt.float32, name="ps", tag="ps")
                    for fc in range(4):
                        nc.tensor.matmul(ps[:], outTn[fc][:, ts(tcn, 128)],
                                         wo_t[fc][:, ts(oc, 512)],
                                         start=(fc == 0), stop=(fc == 3))
                    ob = att.tile([128, 512], dt.float32, name="ob", tag="ob")
                    nc.scalar.copy(ob[:], ps[:])
                    nc.sync.dma_start(out[ts(tcn, 128), ts(oc, 512)], ob[:])
    nc.finalize()
    return nc


def _host_inputs(x, wq, bq, wk, bk, wv, bv, wo, bo):
    """Build the 8 per-core input dicts."""
    r = np.arange(128)[:, None]
    # stripb [128, 384]: band core, stripb[r, u] = |r + 128 - u| <= LK
    u = np.arange(384)[None, :]
    stripb = _bf16((np.abs(r + 128 - u) <= LK).astype(np.float32))
    # strip0 [128, 3968]: band at |r + 1920 - c| <= LK (hg1) or all ones (hg0)
    c = np.arange(3968)[None, :]
    strip0_band = _bf16((np.abs(r + 1920 - c) <= LK).astype(np.float32))
    strip0_ones = _bf16(np.ones((128, 3968), np.float32))

    in_maps = []
    for core in range(NCORES):
        b, hg = core // 2, core % 2
        fsl = slice(hg * FG, (hg + 1) * FG)
        in_maps.append({
            "xT": _bf16(x[b].T),
            "wqT": _bf16(wq[fsl].T),
            "wkT": _bf16(wk[fsl].T),
            "wvT": _bf16(wv[fsl].T),
            "woT": _bf16(wo[:, fsl].T),
            "bqc": np.ascontiguousarray(bq[fsl].reshape(4, 128).T, np.float32),
            "strip0": strip0_ones if hg == 0 else strip0_band,
            "stripb": stripb,
        })
    return in_maps


def kernel(x, wq, bq, wk, bk, wv, bv, wo, bo):
    from concourse.bass_utils import run_bass_kernel_spmd

    x, wq, bq, wk, bk, wv, bv, wo, bo = (
        np.asarray(a, np.float32) for a in (x, wq, bq, wk, bk, wv, bv, wo, bo))

    if "nc" not in _cache:
        _cache["nc"] = _build()
    nc = _cache["nc"]

    in_maps = _host_inputs(x, wq, bq, wk, bk, wv, bv, wo, bo)
    res = run_bass_kernel_spmd(nc, in_maps, core_ids=list(range(NCORES)))
    _cache["last_results"] = res

    const = (bo + bv @ wo.T).astype(np.float32)        # [1024]
    out = np.empty((B, N, E), np.float32)
    for b in range(B):
        out[b] = res.results[2 * b]["out"] + res.results[2 * b + 1]["out"] + const
    return out

